# revision 1
# baseline (speedup 1.0000x reference)
"""Single-head attention (b=4, s=4096, d_embed=1024, d_head=128) on 8 TRN2 NeuronCores.

Sharding: core c -> (batch b = c//2, query-half h = c%2). Each core computes
Q for its 2048-query half and K/V for the full 4096-key sequence of its batch
(K/V projection duplicated across the pair -> no collectives needed).

Device layout trick: host pre-transposes x to x^T [d_embed, seq] (bf16) with the
core's own query-half first in the seq order, so the SPMD graph can use
compile-time offsets. Softmax over keys is order-invariant, so permuting the
key order per-core is harmless.

Softmax trick: scores here are tiny (|s*scale| < ~0.1), so no max-subtraction is
needed. exp'd scores are kept transposed (keys on partitions); the PV matmul
uses exp(S^T) tiles as the stationary operand and V augmented with a ones
column as the moving operand, so the softmax denominators fall out of the same
matmul as column 128 of the output. A per-partition reciprocal multiply
finishes the softmax.
"""

import sys

if "/opt/trn_rl_repo" not in sys.path:
    sys.path.insert(0, "/opt/trn_rl_repo")

import numpy as np
import ml_dtypes

B, S, D, H = 4, 4096, 1024, 128
QS = S // 2          # per-core query rows
NCORES = 8
P = 128
EO = D // P          # 8 embed chunks
KT = S // P          # 32 key tiles
QT = QS // P         # 16 query tiles per core
SCALE = float(1.0 / (np.sqrt(H) * np.sqrt(D)))

_STATE = {}


def _build():
    import concourse.bass as bass  # noqa: F401
    import concourse.mybir as mybir
    import concourse.tile as tile
    from concourse import bacc

    BF16 = mybir.dt.bfloat16
    F32 = mybir.dt.float32

    nc = bacc.Bacc("TRN2", target_bir_lowering=False, debug=False, num_devices=NCORES)

    xT_d = nc.dram_tensor("xT", [D, S], BF16, kind="ExternalInput")
    wq_d = nc.dram_tensor("wqT", [D, H], BF16, kind="ExternalInput")
    wk_d = nc.dram_tensor("wkT", [D, H], BF16, kind="ExternalInput")
    wv_d = nc.dram_tensor("wvT", [D, H], BF16, kind="ExternalInput")
    out_d = nc.dram_tensor("out", [QS, H], F32, kind="ExternalOutput")

    Exp = mybir.ActivationFunctionType.Exp

    with tile.TileContext(nc) as tc:
        with tc.tile_pool(name="persist", bufs=1) as persist:
            wq_sb = persist.tile([P, EO, H], BF16)
            wk_sb = persist.tile([P, EO, H], BF16)
            wv_sb = persist.tile([P, EO, H], BF16)
            qt_sb = persist.tile([P, QS], BF16)        # Q^T [head, q]
            kt_sb = persist.tile([P, S], BF16)         # K^T [head, k]
            vp_sb = persist.tile([P, KT, H + 1], BF16)  # V' [k, head | ones]

            nc.sync.dma_start(wq_sb[:], wq_d.rearrange("(eo p) h -> p eo h", p=P))
            nc.sync.dma_start(wk_sb[:], wk_d.rearrange("(eo p) h -> p eo h", p=P))
            nc.sync.dma_start(wv_sb[:], wv_d.rearrange("(eo p) h -> p eo h", p=P))
            nc.vector.memset(vp_sb[:, :, H : H + 1], 1.0)

            # ---- Phase A: projections (x^T resident) ----
            with (
                tc.tile_pool(name="xpool", bufs=1) as xpool,
                tc.tile_pool(name="psA", bufs=2, space="PSUM") as psA,
            ):
                x_sb = xpool.tile([P, EO, S], BF16)
                x_src = xT_d.rearrange("(eo p) s -> p eo s", p=P)
                for e in range(EO):
                    nc.sync.dma_start(x_sb[:, e, :], x_src[:, e, :])

                # Q^T [head, q]: lhsT = WqT chunk, rhs = x^T chunk (first QS cols)
                for nch in range(QS // 512):
                    ps = psA.tile([P, 512], F32, tag="psA")
                    for e in range(EO):
                        nc.tensor.matmul(
                            ps[:],
                            wq_sb[:, e, :],
                            x_sb[:, e, nch * 512 : (nch + 1) * 512],
                            start=(e == 0),
                            stop=(e == EO - 1),
                        )
                    nc.vector.tensor_copy(qt_sb[:, nch * 512 : (nch + 1) * 512], ps[:])

                # K^T [head, k] over full seq
                for nch in range(S // 512):
                    ps = psA.tile([P, 512], F32, tag="psA")
                    for e in range(EO):
                        nc.tensor.matmul(
                            ps[:],
                            wk_sb[:, e, :],
                            x_sb[:, e, nch * 512 : (nch + 1) * 512],
                            start=(e == 0),
                            stop=(e == EO - 1),
                        )
                    nc.vector.tensor_copy(kt_sb[:, nch * 512 : (nch + 1) * 512], ps[:])

                # V [k, head]: lhsT = x^T seq-tile (stationary), rhs = WvT chunk
                for k4 in range(KT // 4):
                    ps = psA.tile([P, 512], F32, tag="psA")
                    for j in range(4):
                        kt = k4 * 4 + j
                        for e in range(EO):
                            nc.tensor.matmul(
                                ps[:, j * H : (j + 1) * H],
                                x_sb[:, e, kt * P : (kt + 1) * P],
                                wv_sb[:, e, :],
                                start=(e == 0),
                                stop=(e == EO - 1),
                            )
                    nc.vector.tensor_copy(
                        vp_sb[:, k4 * 4 : (k4 + 1) * 4, 0:H],
                        ps.rearrange("p (j h) -> p j h", j=4),
                    )

            # ---- Phase B: scores^T + exp ----
            with tc.tile_pool(name="expp", bufs=1) as expp:
                exp_sb = expp.tile([P, KT, QS], BF16)  # exp(S^T) [k, q]

                with tc.tile_pool(name="psB", bufs=2, space="PSUM") as psB:
                    for kt in range(KT):
                        ps = psB.tile([P, QS], F32, tag="psB")
                        for qch in range(QS // 512):
                            nc.tensor.matmul(
                                ps[:, qch * 512 : (qch + 1) * 512],
                                kt_sb[:, kt * P : (kt + 1) * P],
                                qt_sb[:, qch * 512 : (qch + 1) * 512],
                                start=True,
                                stop=True,
                            )
                        nc.scalar.activation(exp_sb[:, kt, :], ps[:], Exp, scale=SCALE)

                # ---- Phase C: PV + fused softmax denominators ----
                with (
                    tc.tile_pool(name="psC", bufs=6, space="PSUM") as psC,
                    tc.tile_pool(name="outp", bufs=3) as outp,
                ):
                    for qt in range(QT):
                        po = psC.tile([P, H + 1], F32, tag="psC")
                        for kt in range(KT):
                            nc.tensor.matmul(
                                po[:],
                                exp_sb[:, kt, qt * P : (qt + 1) * P],
                                vp_sb[:, kt, :],
                                start=(kt == 0),
                                stop=(kt == KT - 1),
                            )
                        rec = outp.tile([P, 1], F32, tag="rec")
                        nc.vector.reciprocal(rec[:], po[:, H : H + 1])
                        ot = outp.tile([P, H], F32, tag="ot")
                        nc.vector.tensor_scalar_mul(ot[:], po[:, 0:H], rec[:])
                        nc.sync.dma_start(out_d[qt * P : (qt + 1) * P, :], ot[:])

    nc.compile()
    return nc


def _get_nc():
    if "nc" not in _STATE:
        _STATE["nc"] = _build()
    return _STATE["nc"]


def _make_in_maps(x, Wq, Wk, Wv):
    bf16 = ml_dtypes.bfloat16
    wq = np.ascontiguousarray(np.asarray(Wq).T).astype(bf16)
    wk = np.ascontiguousarray(np.asarray(Wk).T).astype(bf16)
    wv = np.ascontiguousarray(np.asarray(Wv).T).astype(bf16)
    x = np.asarray(x)
    in_maps = []
    for c in range(NCORES):
        b, h = divmod(c, 2)
        xb = x[b]
        xperm = np.concatenate([xb[h * QS : (h + 1) * QS], xb[(1 - h) * QS : (2 - h) * QS]], axis=0)
        xT = np.ascontiguousarray(xperm.T).astype(bf16)
        in_maps.append({"xT": xT, "wqT": wq, "wkT": wk, "wvT": wv})
    return in_maps


def _assemble(results):
    out = np.empty((B, S, H), np.float32)
    for c in range(NCORES):
        b, h = divmod(c, 2)
        out[b, h * QS : (h + 1) * QS, :] = results[c]["out"]
    return out


def run(x, Wq, Wk, Wv, trace=False, trace_cores=None):
    """Run on HW; returns (output, BassKernelResults)."""
    from concourse.bass_utils import run_bass_kernel_spmd

    nc = _get_nc()
    in_maps = _make_in_maps(x, Wq, Wk, Wv)
    res = run_bass_kernel_spmd(
        nc,
        in_maps,
        list(range(NCORES)),
        trace=trace,
        trace_cores=trace_cores,
    )
    return _assemble(res.results), res


def kernel(x, Wq, Wk, Wv):
    out, _ = run(x, Wq, Wk, Wv)
    return out
